# revision 23
# baseline (speedup 1.0000x reference)
"""Distributed single-head attention for Trainium2 (8 NeuronCores).

Problem: B=4, S=2048, D=1024 fp32 attention:
    q = x@Wq+bq; k = x@Wk+bk; v = x@Wv+bv
    out = softmax(q k^T / sqrt(D) + mask) v

Sharding: data-parallel over (batch, query-half): core c handles batch
c//2, query rows [1024*(c%2), +1024) with the full key range; no
on-chip collectives. Host precomputes fold the K projection away
(M2 = Wq Wk^T, w2 = Wk bq; the per-query constant drops in softmax)
and defer the V projection to the output side (out = (A_norm @ x) Wv
+ bv), so the PE work per core is TT = x_q M2 (hybrid fp8/bf16),
scores = TT x^T (fp8 DoubleRow), PV' = A_norm x (fp16), O = PV' Wv
(fp16). The first 768 of TT's 1024 contraction features run in fp8
DoubleRow; the value path (xn/exp/PV'/Wv/out) is fp16, which buys the
error margin for the 768 split (emulated+measured rel err 1.953e-2 vs
the 2e-2 gate; the numpy emulator in emu.py reproduces hardware to 4
digits).

Schedule per iteration (PE): scores(qc+1), PV'(qc), O(qc-1) — O always
trails its PV' by a full iteration minus one phase so the pvn-evict +
pvt-transpose chain has ~10us of slack, and at(qc) was DMA-transposed
during iteration qc-1 right after each exp half (h0 on scalar ring, h1
on sync) so PV'(qc) never waits. Iteration 0 has no O phase; 10
dependency-free filler matmuls keep the PE p-state hot while
exp(0)+at(0) complete (an idle PE drops to half clock and pays ~8
half-speed matmuls on resume — measured). 28 warmup matmuls cover the
engine-boot + first-load window for the same reason. The final O chunk
evicts+stores in quarters to shorten the kernel tail.

NOTE: loading x8 as split query/key-half DMAs into the same tile
corrupts the DoubleRow rhs reads (rel err jumps to ~7e-2 on hardware);
keep the full-row x8 loads.
"""

from contextlib import ExitStack

import numpy as np
import ml_dtypes

import concourse.tile as tile
import concourse.mybir as mybir
from concourse import bacc
from concourse.bass_utils import run_bass_kernel_spmd

BF16 = mybir.dt.bfloat16
F16 = mybir.dt.float16
F32 = mybir.dt.float32
F8 = mybir.dt.float8e4
AF = mybir.ActivationFunctionType

D = 1024
S = 2048
Q = 1024
P = 128
ND = D // P
NS = S // P
NQ = Q // P
F8_FEATS = 768
NG = F8_FEATS // (2 * P)
NBF = (D - F8_FEATS) // P
SCALE = 1.0 / float(np.sqrt(np.float32(D)))

_NC_CACHE: dict[bool, bacc.Bacc] = {}


def _build(use_mask: bool) -> bacc.Bacc:
    nc = bacc.Bacc("TRN2", target_bir_lowering=False, debug=False, num_devices=8)

    x8_d = nc.dram_tensor("x8", [D, S], F8, kind="ExternalInput")
    xq_d = nc.dram_tensor("xq", [D - F8_FEATS, Q], BF16, kind="ExternalInput")
    m2_d = nc.dram_tensor("m2", [D - F8_FEATS, D], BF16, kind="ExternalInput")
    m28_d = nc.dram_tensor("m28", [F8_FEATS, D], F8, kind="ExternalInput")
    xn_d = nc.dram_tensor("xn", [S, D], F16, kind="ExternalInput")
    wv_d = nc.dram_tensor("wv", [D, D], F16, kind="ExternalInput")
    w2_d = nc.dram_tensor("w22", [P, ND], F32, kind="ExternalInput")
    bv_d = nc.dram_tensor("bvr", [1, D], F16, kind="ExternalInput")
    if use_mask:
        mask_d = nc.dram_tensor("maskp", [Q, S], F32, kind="ExternalInput")
    out_d = nc.dram_tensor("out", [Q, D], F16, kind="ExternalOutput")

    with tile.TileContext(nc) as tc, ExitStack() as ctx:
        x8_pool = ctx.enter_context(tc.tile_pool(name="x8", bufs=ND // 2))
        m2_pool = ctx.enter_context(tc.tile_pool(name="m2", bufs=NG + 2))
        wv_pool = ctx.enter_context(tc.tile_pool(name="wv", bufs=2))
        xn_pool = ctx.enter_context(tc.tile_pool(name="xn", bufs=2))
        tt_pool = ctx.enter_context(tc.tile_pool(name="tt", bufs=ND // 2))
        const_pool = ctx.enter_context(tc.tile_pool(name="const", bufs=1))
        exp_pool = ctx.enter_context(tc.tile_pool(name="exp", bufs=2))
        at_pool = ctx.enter_context(tc.tile_pool(name="at", bufs=2))
        pvn_pool = ctx.enter_context(tc.tile_pool(name="pvn", bufs=2))
        pvt_pool = ctx.enter_context(tc.tile_pool(name="pvt", bufs=2))
        stat_pool = ctx.enter_context(tc.tile_pool(name="stat", bufs=8))
        o_pool = ctx.enter_context(tc.tile_pool(name="o", bufs=2))
        if use_mask:
            m_pool = ctx.enter_context(tc.tile_pool(name="m", bufs=2))
        psum = ctx.enter_context(tc.tile_pool(name="psum", bufs=4, space="PSUM"))

        xa8 = [
            x8_pool.tile([P, 2 * S], F8, tag="xa8", name=f"xa8_{i}")
            for i in range(ND // 2)
        ]
        m28 = [
            m2_pool.tile([P, 2 * D], F8, tag="m28", name=f"m28_{i}")
            for i in range(NG)
        ]
        m2_sb = m2_pool.tile([P, NBF * D], BF16, tag="m2")
        xq_sb = m2_pool.tile([P, NBF * Q], BF16, tag="xq")
        wv = [
            wv_pool.tile([P, 4 * D], F16, tag="wv", name=f"wv{i}") for i in range(2)
        ]
        xn = [
            xn_pool.tile([P, 8 * D], F16, tag="xn", name=f"xn{i}") for i in range(2)
        ]
        w2_sb = const_pool.tile([P, ND], F32, tag="w2")
        bvb_sb = const_pool.tile([P, D], F16, tag="bvb")

        warm_in = const_pool.tile([P, 512], BF16, tag="warmin")
        nc.vector.memset(warm_in[:], 0.001)
        warm = psum.tile([P, Q], F32, tag="ps", name="warm")
        for _ in range(28):
            nc.tensor.matmul(
                warm[:, 0:512],
                lhsT=warm_in[:, 0:P],
                rhs=warm_in[:, 0:512],
                start=True,
                stop=True,
            )
        m28_src = m28_d.ap().rearrange("(g i p) e -> g p i e", g=NG, i=2, p=P)
        for g in range(NG):
            nc.scalar.dma_start(
                m28[g].rearrange("p (i e) -> p i e", i=2), m28_src[g]
            )
        nc.scalar.dma_start(
            xq_sb.rearrange("p (c q) -> p c q", c=NBF),
            xq_d.ap().rearrange("(c p) q -> p c q", c=NBF, p=P),
        )
        nc.scalar.dma_start(
            m2_sb.rearrange("p (c e) -> p c e", c=NBF),
            m2_d.ap().rearrange("(c p) e -> p c e", c=NBF, p=P),
        )
        nc.scalar.dma_start(w2_sb[:], w2_d[:, :])
        nc.scalar.dma_start(bvb_sb[:], bv_d[0:1, :].partition_broadcast(P))
        for d in range(ND):
            g, i = divmod(d, 2)
            nc.sync.dma_start(
                xa8[g][:, i * S : (i + 1) * S], x8_d[d * P : (d + 1) * P, :]
            )
        xn_src = xn_d.ap().rearrange("(i k p) d -> i p k d", i=2, p=P)
        for i in range(2):
            nc.sync.dma_start(xn[i].rearrange("p (k d) -> p k d", k=8), xn_src[i])

        tt8 = [
            tt_pool.tile([P, 2 * Q], F8, tag="tt", name=f"tt8_{i}")
            for i in range(ND // 2)
        ]
        for eb in range(2):
            for j in range(4):
                e = eb * 4 + j
                ps = psum.tile([P, Q], F32, tag="ps", name=f"tps{e}")
                for g in range(NG):
                    lt = m28[g].rearrange("p (i e) -> p i e", i=2)[
                        :, :, e * P : (e + 1) * P
                    ]
                    for n in range(2):
                        rh = xa8[g].rearrange("p (i s) -> p i s", i=2)[
                            :, :, n * 512 : (n + 1) * 512
                        ]
                        nc.tensor.matmul(
                            ps[:, n * 512 : (n + 1) * 512],
                            lhsT=lt,
                            rhs=rh,
                            start=(g == 0),
                            stop=False,
                            perf_mode=mybir.MatmulPerfMode.DoubleRow,
                        )
                for c in range(NBF):
                    lt2 = m2_sb.rearrange("p (c e) -> p c e", c=NBF)[
                        :, c, e * P : (e + 1) * P
                    ]
                    for n in range(2):
                        rh2 = xq_sb.rearrange("p (c q) -> p c q", c=NBF)[
                            :, c, n * 512 : (n + 1) * 512
                        ]
                        nc.tensor.matmul(
                            ps[:, n * 512 : (n + 1) * 512],
                            lhsT=lt2,
                            rhs=rh2,
                            start=False,
                            stop=(c == NBF - 1),
                        )
                dst = tt8[e // 2][:, (e % 2) * Q : (e % 2) * Q + Q]
                if j % 2 == 0:
                    nc.scalar.activation(
                        dst, ps[:], AF.Identity, bias=w2_sb[:, e : e + 1]
                    )
                else:
                    nc.vector.tensor_scalar_add(dst, ps[:], w2_sb[:, e : e + 1])

        def scores_mm(qc):
            pss = []
            for half in range(2):
                ps = psum.tile([P, Q], F32, tag="ps", name=f"sps{qc}_{half}")
                for g in range(ND // 2):
                    lt = tt8[g].rearrange("p (i q) -> p i q", i=2)[
                        :, :, qc * P : (qc + 1) * P
                    ]
                    for n in range(2):
                        off = half * 1024 + n * 512
                        rh = xa8[g].rearrange("p (i s) -> p i s", i=2)[
                            :, :, off : off + 512
                        ]
                        nc.tensor.matmul(
                            ps[:, n * 512 : (n + 1) * 512],
                            lhsT=lt,
                            rhs=rh,
                            start=(g == 0),
                            stop=(g == ND // 2 - 1),
                            perf_mode=mybir.MatmulPerfMode.DoubleRow,
                        )
                pss.append(ps)
            return pss

        def exp_at_stats(qc, pss):
            sums = stat_pool.tile([P, 2], F32, tag="sums", name=f"sums{qc}")
            ats = []
            for half in range(2):
                ps = pss[half]
                if use_mask:
                    mt = m_pool.tile([P, Q], F32, tag="m", name=f"mt{qc}_{half}")
                    nc.sync.dma_start(
                        mt[:],
                        mask_d[qc * P : (qc + 1) * P, half * 1024 : (half + 1) * 1024],
                    )
                    nc.vector.tensor_add(ps[:], ps[:], mt[:])
                exp_sb = exp_pool.tile(
                    [P, Q], F16, tag=f"exp{half}", name=f"exp{qc}_{half}"
                )
                nc.scalar.activation(
                    exp_sb[:],
                    ps[:],
                    AF.Exp,
                    scale=SCALE,
                    accum_out=sums[:, half : half + 1],
                )
                at_sb = at_pool.tile(
                    [P, Q], F16, tag=f"at{half}", name=f"at{qc}_{half}"
                )
                ring = nc.scalar if half == 0 else nc.sync
                ring.dma_start(
                    out=at_sb.rearrange("p (c q) -> p c q", q=P),
                    in_=exp_sb[:],
                    transpose=True,
                )
                ats.append(at_sb)
            rsum = stat_pool.tile([P, 1], F32, tag="rsum", name=f"rsum{qc}")
            nc.vector.tensor_add(rsum[:], sums[:, 0:1], sums[:, 1:2])
            rinv = stat_pool.tile([P, 1], F32, tag="rinv", name=f"rinv{qc}")
            nc.vector.reciprocal(rinv[:], rsum[:])
            return ats, rinv

        def pv_mm(qc, ats, rinv):
            pv = psum.tile([P, D], F32, tag="ps", name=f"pv{qc}")
            for k in range(NS):
                for n in range(2):
                    nc.tensor.matmul(
                        pv[:, n * 512 : (n + 1) * 512],
                        lhsT=ats[k // 8][:, (k % 8) * P : (k % 8 + 1) * P],
                        rhs=xn[k // 8][
                            :, (k % 8) * 1024 + n * 512 : (k % 8) * 1024 + (n + 1) * 512
                        ],
                        start=(k == 0),
                        stop=(k == NS - 1),
                    )
            pvn = pvn_pool.tile([P, D], F16, tag="pvn", name=f"pvn{qc}")
            pvt = pvt_pool.tile([P, D], F16, tag="pvt", name=f"pvt{qc}")
            for hh in range(2):
                sl = slice(hh * 512, (hh + 1) * 512)
                nc.vector.tensor_scalar_mul(pvn[:, sl], pv[:, sl], rinv[:])
                ring = nc.scalar if hh == 0 else nc.sync
                ring.dma_start(
                    out=pvt[:, sl].rearrange("p (c q) -> p c q", q=P),
                    in_=pvn[:, sl],
                    transpose=True,
                )
            return pvt

        def o_phase(qc, pvt):
            op = psum.tile([P, D], F32, tag="ps", name=f"op{qc}")
            for dc in range(ND):
                for n in range(2):
                    nc.tensor.matmul(
                        op[:, n * 512 : (n + 1) * 512],
                        lhsT=pvt[:, dc * P : (dc + 1) * P],
                        rhs=wv[dc // 4][
                            :, (dc % 4) * 1024 + n * 512 : (dc % 4) * 1024 + (n + 1) * 512
                        ],
                        start=(dc == 0),
                        stop=(dc == ND - 1),
                    )
            oo = o_pool.tile([P, D], F16, tag="o", name=f"oo{qc}")
            if qc == NQ - 1:
                for hh in range(4):
                    sl = slice(hh * 256, (hh + 1) * 256)
                    nc.vector.tensor_add(oo[:, sl], op[:, sl], bvb_sb[:, sl])
                    nc.sync.dma_start(out_d[qc * P : (qc + 1) * P, sl], oo[:, sl])
            else:
                nc.vector.tensor_add(oo[:], op[:], bvb_sb[:, :])
                nc.sync.dma_start(out_d[qc * P : (qc + 1) * P, :], oo[:])

        cur_at, cur_rinv = exp_at_stats(0, scores_mm(0))
        prev_pvt = None
        for qc in range(NQ):
            if qc + 1 < NQ:
                pss = scores_mm(qc + 1)
                nxt_at, nxt_rinv = exp_at_stats(qc + 1, pss)
                if qc == 0:
                    wv_src = wv_d.ap().rearrange(
                        "(i c p) e -> i p c e", i=2, c=4, p=P
                    )
                    for i in range(2):
                        nc.sync.dma_start(
                            wv[i].rearrange("p (c e) -> p c e", c=4), wv_src[i]
                        )
                    warm2 = psum.tile([P, Q], F32, tag="ps", name="warm2")
                    for _ in range(10):
                        nc.tensor.matmul(
                            warm2[:, 0:512],
                            lhsT=warm_in[:, 0:P],
                            rhs=warm_in[:, 0:512],
                            start=True,
                            stop=True,
                        )
                prev_pvt_new = pv_mm(qc, cur_at, cur_rinv)
                if prev_pvt is not None:
                    o_phase(qc - 1, prev_pvt)
                prev_pvt = prev_pvt_new
                cur_at, cur_rinv = nxt_at, nxt_rinv
            else:
                cur = pv_mm(qc, cur_at, cur_rinv)
                o_phase(qc - 1, prev_pvt)
                prev_pvt = cur
        o_phase(NQ - 1, prev_pvt)

    nc.compile()
    return nc


def _get_nc(use_mask: bool) -> bacc.Bacc:
    if use_mask not in _NC_CACHE:
        _NC_CACHE[use_mask] = _build(use_mask)
    return _NC_CACHE[use_mask]


def kernel(x, mask, Wq, bq, Wk, bk, Wv, bv):
    x = np.asarray(x, dtype=np.float32)
    mask = np.asarray(mask, dtype=np.float32)
    Wq = np.asarray(Wq, dtype=np.float32)
    bq = np.asarray(bq, dtype=np.float32)
    Wk = np.asarray(Wk, dtype=np.float32)
    bk = np.asarray(bk, dtype=np.float32)
    Wv = np.asarray(Wv, dtype=np.float32)
    bv = np.asarray(bv, dtype=np.float32)

    B = x.shape[0]
    use_mask = bool(np.any(mask))
    nc = _get_nc(use_mask)

    bf = ml_dtypes.bfloat16
    m2 = (Wq.astype(np.float64) @ Wk.astype(np.float64).T).astype(bf)
    m28 = m2[0:F8_FEATS, :].astype(ml_dtypes.float8_e4m3)
    m2t = np.ascontiguousarray(m2[F8_FEATS:, :])
    w2 = (Wk.astype(np.float64) @ bq.astype(np.float64)).astype(np.float32)
    w22 = np.ascontiguousarray(w2.reshape(ND, P).T)
    wv_h = Wv.astype(np.float16)
    bvr = bv.reshape(1, D).astype(np.float16)

    in_maps = []
    for c in range(8):
        b, h = divmod(c, 2)
        xp = np.concatenate(
            [x[b, h * Q : (h + 1) * Q], x[b, (1 - h) * Q : (2 - h) * Q]]
        )
        xn = np.ascontiguousarray(xp).astype(np.float16)
        xpt = np.ascontiguousarray(xp.T)
        im = {
            "x8": xpt.astype(ml_dtypes.float8_e4m3),
            "xq": np.ascontiguousarray(xpt[F8_FEATS:, 0:Q]).astype(bf),
            "m2": m2t,
            "m28": m28,
            "xn": xn,
            "wv": wv_h,
            "w22": w22,
            "bvr": bvr,
        }
        if use_mask:
            mrows = mask[h * Q : (h + 1) * Q]
            mperm = np.concatenate(
                [mrows[:, h * Q : (h + 1) * Q], mrows[:, (1 - h) * Q : (2 - h) * Q]],
                axis=1,
            )
            im["maskp"] = np.ascontiguousarray(mperm / np.float32(SCALE)).astype(
                np.float32
            )
        in_maps.append(im)

    res = run_bass_kernel_spmd(nc, in_maps, core_ids=list(range(8)))

    out = np.empty((B, S, D), dtype=np.float32)
    for c in range(8):
        b, h = divmod(c, 2)
        out[b, h * Q : (h + 1) * Q, :] = res.results[c]["out"].astype(np.float32)
    return out
